# revision 39
# baseline (speedup 1.0000x reference)
"""Two-layer GRU (16->128->128) + FC(128->24) head on 8 Trainium2 NeuronCores.

Strategy: data-parallel over the batch (4096 -> 512 per core); tiny weights
replicated. On each core the hidden state lives transposed in SBUF as
[H=128 partitions, B=512 free]. Per time step, gate pre-activations are
accumulated in PSUM by matmuls (input-projection + recurrent + biases
folded in), sigmoid/tanh run on the scalar engine, and the cell update is
spread across vector + gpsimd engines.

Execution path: one cached jit(shard_map) callable whose body is ONLY the
bass_exec custom call; the host repacks x to the time-major transposed
[L, 17, BL] f16 layout, device_puts pre-sharded arguments, and gathers the
small [24, 512]-per-core output. The two GRU layers are software-pipelined
(layer 2 one step behind layer 1) so the per-step serial chain is a single
layer's, and the h-update uses h' = (1-z)*n + z*h with the (1-z) and z*h
pieces precomputed off the critical path. The r-gate recurrent matmul
streams (u, zh) as two rhs tensors (U@u + U@zh == U@h') so the chain skips
the final h' = u + zh add, and imm1's position in the PE stream is tuned
so it issues right as t2 lands. (Pool-engine offload of z*h and a joint
cross-layer z-sigmoid are implemented behind CONFIG flags but measured
slower on HW / in TimelineSim, so they default off.)

Self-contained: hardcodes all shapes; no file I/O.
"""

import numpy as np

import bass_rust
import concourse.bass as bass
import concourse.mybir as mybir
from concourse.tile import TileContext
from concourse.vector_clock import ScopedClock

N_CORES = 8
B_TOT = 4096
L = 128          # sequence length (= 2*1024/16)
D = 16           # per-step input features
DA = 17          # + ones row (bias folding for layer 1)
H = 128          # hidden
G3 = 3 * H       # 384 stacked gates (r, z, n)
BL = B_TOT // N_CORES  # 512 batch per core
NCLS = 24
CHUNK = 8        # time steps of x staged into SBUF per DMA

F32 = mybir.dt.float32
F32R = mybir.dt.float32r
F16 = mybir.dt.float16
BF16 = mybir.dt.bfloat16
AF = mybir.ActivationFunctionType
OP = mybir.AluOpType

# Tunables (grid-searched via TimelineSim, validated on HW).
CONFIG = {
    "dtype": "f16",      # gate/h/weight/x dtype: "f32r" | "f16" | "bf16"
    "zh1": True,         # L1 h-update via oz/zh trick (2 post-tanh hops)
    "zh2": True,         # L2 h-update likewise (else d/e/h chain)
    "mm_order": "rz_first",  # prz matmul order: "rz_first" | "r_first"
    "t22_late": False,   # emit layer-2 t2/imm after layer-1's h update
    "t2_early": False,   # emit layer-1 t2 right after its sigmoids
    "prep_late": False,  # emit oz/zh after t2/imm instead of after sig
    "prep1_eng": "v",    # engine for layer-1 oz/zh
    "prep2_eng": "v",    # engine for layer-2 oz/zh
    "t2_eng": "v",       # engine for t2 = (pn + b) * r
    "upd1_eng": "v",     # engine for layer-1 u / h'
    "oz1_eng": "v",      # engine for layer-1 oz = 1 - z
    "oz2_eng": "v",      # engine for layer-2 oz = 1 - z
    "upd2_eng": "v",     # engine for layer-2 u / h' 
    "rz1_fused": False,  # one sigmoid over [H, 2BL] for layer-1 r|z
    "rz2_fused": False,  # (needs PSUM bias; only valid when rz_bias is None)
    "h_bufs": 2,
    "work_bufs": 3,
    "sr1_psum": False,   # layer-1 r sigmoid written in-place into PSUM
    "sr2_psum": False,   # layer-2 r sigmoid written in-place into PSUM
    "rmm1_double": True,  # L1 r-gate rec mm streams (u, zh) instead of h'
    "rmm2_double": True,  # L2 likewise (skips the u+zh add on the r path)
    "l2_lag": 1,         # layer-2 pipeline lag in steps (1 or 2)
    "imm1_pos": 4,       # of L2's rz matmuls emitted before imm1 on the PE
    "tanh1_before_sig2": False,  # Act order: tanh1 before (sr2, sz2)
    "t2_split": False,   # t2 + imm in two half-width wavefront stages
    "szj": False,        # one sigmoid over [z1|z2] (joint PSUM layout
                         # [r1|z1|z2|r2]; z2 bias via rank-1 PE matmul)
    "szj_first": False,  # Act order szj before sr2 (else after)
    "tail2_defer": False,  # emit L2's tanh/update at the top of the NEXT
                           # iteration so tanh2 doesn't queue-block sr1
}

_DT = {"f32r": F32R, "f16": F16, "bf16": BF16}
_NP_DT = {"f32r": np.float32, "f16": np.float16}


class SplitDrainTileContext(TileContext):
    """Walrus (CoreV3) rejects instructions carrying >2 sync waits; Tile's
    kernel-tail drain accumulates one wait per outstanding engine/DMA-queue
    sem. Split them across a chain of drains (1 wait each)."""

    def _drain_and_barrier(self, tick_clock, wait_clock):
        nc = self.nc
        drain_inst = nc.sync.drain()
        wait_clock.add_sem_waits(
            drain_inst.ins, ScopedClock({None: tick_clock.global_clock})
        )
        si = drain_inst.ins.sync_info
        if si is not None and len(si.on_wait) > 1:
            waits = list(si.on_wait)
            si.on_wait = waits[:1]
            for w in waits[1:]:
                d2 = nc.sync.drain()
                d2.ins.sync_info = bass_rust.SyncInfo(on_wait=[w], on_update=[])
        nc.all_engine_barrier()
        popped = nc._tile_sem_poison_stack.pop()
        assert popped is self._sem_poison
        nc.clear_and_free_semaphores(list(self.sems.allocated().values()))
        nc.all_engine_barrier()


def _split_excess_waits(nc: bass.Bass, max_waits: int = 1) -> None:
    """Walrus (CoreV3 setupSyncWait) accepts at most 2 sem waits per
    instruction; Tile occasionally attaches 3+. Hoist the excess onto
    EventSemaphore instructions inserted right before the offender on the
    same engine (serial waits AND together)."""
    n = 0
    for fn in nc.m.functions:
        for bb in fn.blocks:
            out = []
            dirty = False
            for inst in bb.instructions:
                si = inst.sync_info
                if si is not None and len(si.on_wait) > max_waits:
                    waits = list(si.on_wait)
                    extra = waits[: len(waits) - max_waits]
                    for w in extra:
                        ev = mybir.InstEventSemaphore(
                            name=f"evs-waitsplit-{n}", ins=[], outs=[]
                        )
                        n += 1
                        ev.engine = inst.engine
                        ev.sync_info = bass_rust.SyncInfo(
                            on_wait=[w], on_update=[]
                        )
                        out.append(ev)
                    si.on_wait = waits[len(waits) - max_waits :]
                    dirty = True
                out.append(inst)
            if dirty:
                bb.instructions = out


def build_program(for_sim: bool = False, n_steps: int = L,
                  repeats: int = 1) -> bass.Bass:
    # for_sim: skip the walrus wait-limit workarounds (post-hoc IR mutations
    # that CoreSim's bookkeeping doesn't understand); semantics identical.
    # repeats: run the whole forward pass R times back-to-back (timing
    # programs; the marginal wall-clock per extra pass is the HW exec time).
    nc = bass.Bass()

    DT = _DT[CONFIG["dtype"]]
    xT_d = nc.declare_dram_parameter("xT", [L, DA, BL], DT, isOutput=False)
    l1w_d = nc.declare_dram_parameter("l1w", [DA, G3], DT, isOutput=False)
    hh1_d = nc.declare_dram_parameter("hh1w", [H, G3], DT, isOutput=False)
    ih2_d = nc.declare_dram_parameter("ih2w", [H, G3], DT, isOutput=False)
    hh2_d = nc.declare_dram_parameter("hh2w", [H, G3], DT, isOutput=False)
    bias_d = nc.declare_dram_parameter("bvec", [H, 5], F32, isOutput=False)
    fcw_d = nc.declare_dram_parameter("fcw", [H, NCLS], DT, isOutput=False)
    fcb_d = nc.declare_dram_parameter("fcb", [NCLS, 1], F32, isOutput=False)
    ident_d = nc.declare_dram_parameter("ident", [H, H], DT, isOutput=False)
    bz2r_d = nc.declare_dram_parameter("bz2row", [1, H], DT, isOutput=False)
    ones_d = nc.declare_dram_parameter("onesr", [1, BL], DT, isOutput=False)
    out_d = nc.declare_dram_parameter("outT", [NCLS, BL], F32, isOutput=True)

    tc_cls = TileContext if for_sim else SplitDrainTileContext
    with tc_cls(nc) as tc:
        with (
            tc.tile_pool(name="singles", bufs=1) as singles,
            tc.tile_pool(name="xchunks", bufs=3) as xpool,
            tc.tile_pool(name="hstate",
                         bufs=max(CONFIG["h_bufs"],
                                  CONFIG["l2_lag"] + 1)) as hpool,
            tc.tile_pool(name="work", bufs=CONFIG["work_bufs"]) as work,
            tc.tile_pool(name="prz", bufs=1, space="PSUM") as przpool,
            tc.tile_pool(name="pnx", bufs=1, space="PSUM") as pnxpool,
        ):
            # --- constant loads -------------------------------------------
            l1w = singles.tile([DA, G3], DT, tag="l1w")
            hh1w = singles.tile([H, G3], DT, tag="hh1w")
            ih2w = singles.tile([H, G3], DT, tag="ih2w")
            hh2w = singles.tile([H, G3], DT, tag="hh2w")
            sbias = singles.tile([H, 5], F32, tag="sbias")
            fcw = singles.tile([H, NCLS], DT, tag="fcw")
            fcb = singles.tile([NCLS, 1], F32, tag="fcb")
            ident = singles.tile([H, H], DT, tag="ident")
            bz2row = singles.tile([1, H], DT, tag="bz2row")
            onesr = singles.tile([1, BL], DT, tag="onesr")
            nc.sync.dma_start(out=bz2row[:], in_=bz2r_d[:])
            nc.sync.dma_start(out=onesr[:], in_=ones_d[:])
            nc.sync.dma_start(out=ident[:], in_=ident_d[:])
            nc.sync.dma_start(out=l1w[:], in_=l1w_d[:])
            nc.sync.dma_start(out=hh1w[:], in_=hh1_d[:])
            nc.sync.dma_start(out=ih2w[:], in_=ih2_d[:])
            nc.sync.dma_start(out=hh2w[:], in_=hh2_d[:])
            nc.sync.dma_start(out=sbias[:], in_=bias_d[:])
            nc.sync.dma_start(out=fcw[:], in_=fcw_d[:])
            nc.sync.dma_start(out=fcb[:], in_=fcb_d[:])

            ENG = {"v": nc.vector, "g": nc.gpsimd}
            shared = {}

            class Cell:
                """Stage-split GRU cell so the two layers' instruction
                streams can be interleaved (software pipelining: layer 2
                runs one time step behind layer 1). Engines execute their
                streams in order, so emission order determines the
                schedule."""

                def __init__(self, tag, xw, hw, rz_bias, n_hh_bias,
                             n_ih_bias, use_zh):
                    self.tag = tag
                    self.xw = xw            # [K, G3] lhsT for the input proj
                    self.hw = hw            # [H, G3] recurrent lhsT
                    self.rz_bias = rz_bias  # None (folded) or (r_ap, z_ap)
                    self.n_hh_bias = n_hh_bias
                    self.n_ih_bias = n_ih_bias
                    self.use_zh = use_zh
                    # (u, zh) tiles of the last update: with rmm_double the
                    # r-gate recurrent matmul streams these two rhs tensors
                    # (U@u + U@zh == U@h') so it needn't wait for the final
                    # h' = u + zh add on the critical path.
                    self.u_last = None
                    self.zh_last = None

                def _alloc_prz(self):
                    # PSUM gate layout. Default: per-cell [r|z] tile. szj:
                    # one shared joint tile [r1|z1|z2|r2] so the two z
                    # pre-activations sit adjacent for the joint sigmoid.
                    tag = self.tag
                    if not CONFIG["szj"]:
                        self.prz = przpool.tile([H, 2 * BL], F32,
                                                tag=f"prz{tag}")
                        self.prz_r = self.prz[:, 0:BL]
                        self.prz_z = self.prz[:, BL:]
                        self.prz_rz = self.prz[:]
                        return
                    if tag == "1" or not shared.get("przJ_fresh"):
                        shared["przJ"] = przpool.tile([H, 4 * BL], F32,
                                                      tag="przJ",
                                                      name="przJ")
                        shared["przJ_fresh"] = True
                    t = shared["przJ"]
                    self.przJ = t
                    if tag == "1":
                        self.prz_r = t[:, 0:BL]
                        self.prz_z = t[:, BL : 2 * BL]
                        self.prz_rz = t[:, 0 : 2 * BL]
                    else:
                        shared["przJ_fresh"] = False
                        self.prz_z = t[:, 2 * BL : 3 * BL]
                        self.prz_r = t[:, 3 * BL : 4 * BL]

                def _rec_r_mms(self, h_prev):
                    # r-gate recurrent matmul (the chain-critical one).
                    if (CONFIG[f"rmm{self.tag}_double"] and self.use_zh
                            and self.u_last is not None):
                        nc.tensor.matmul(self.prz_r, self.hw[:, 0:H],
                                         self.u_last[:], start=False,
                                         stop=self.zh_last is None)
                        if self.zh_last is not None:
                            nc.tensor.matmul(self.prz_r,
                                             self.hw[:, 0:H],
                                             self.zh_last[:], start=False,
                                             stop=True)
                    else:
                        nc.tensor.matmul(self.prz_r, self.hw[:, 0:H],
                                         h_prev[:], start=False, stop=True)

                def in_mms(self, x_rhs, alone=False):
                    # Input-projection matmuls (only need x / h1): allocate
                    # the PSUM tiles and write the start-of-accumulation
                    # parts. Emitted an iteration EARLY so they don't sit on
                    # the h -> gates critical path.
                    tag = self.tag
                    self._alloc_prz()
                    self.px = pnxpool.tile([H, BL], F32, tag=f"px{tag}")
                    nc.tensor.matmul(self.prz_r, self.xw[:, 0:H],
                                     x_rhs, start=True, stop=alone)
                    nc.tensor.matmul(self.prz_z, self.xw[:, H : 2 * H],
                                     x_rhs, start=True, stop=alone)
                    nc.tensor.matmul(self.px[:], self.xw[:, 2 * H :], x_rhs,
                                     start=True, stop=False)

                def rec_mms(self, h_prev):
                    # Recurrent matmuls: the only PE work on the h critical
                    # path. r first (it gates the sigmoid -> t2 chain).
                    tag = self.tag
                    self.pn = pnxpool.tile([H, BL], F32, tag=f"pn{tag}")
                    self._rec_r_mms(h_prev)
                    nc.tensor.matmul(self.prz_z, self.hw[:, H : 2 * H],
                                     h_prev[:], start=False, stop=True)
                    nc.tensor.matmul(self.pn[:], self.hw[:, 2 * H :],
                                     h_prev[:], start=True, stop=True)

                def stage_mms(self, x_rhs, h_prev):
                    # Input + recurrent matmuls as a staged thunk list so the
                    # loop can interleave other engines' critical
                    # instructions at chosen points of the PE stream.
                    # self._rz holds the r/z-gate matmuls (must all be
                    # emitted before sig()); self._npx holds pn/px.
                    tag = self.tag
                    self._alloc_prz()
                    self.px = pnxpool.tile([H, BL], F32, tag=f"px{tag}")
                    px, xw, hw = self.px, self.xw, self.hw
                    prz_r, prz_z = self.prz_r, self.prz_z
                    alone = h_prev is None
                    # szj: z2's sigmoid bias lands in PSUM via a rank-1
                    # matmul so the joint (bias-free) sigmoid stays valid.
                    zbias_mm = CONFIG["szj"] and tag == "2"
                    rz = [lambda: nc.tensor.matmul(prz_r, xw[:, 0:H],
                                                   x_rhs, start=True,
                                                   stop=alone)]
                    if not alone:
                        if (CONFIG[f"rmm{tag}_double"] and self.use_zh
                                and self.u_last is not None):
                            u, zh = self.u_last, self.zh_last
                            rz.append(lambda: nc.tensor.matmul(
                                prz_r, hw[:, 0:H], u[:],
                                start=False, stop=zh is None))
                            if zh is not None:
                                rz.append(lambda: nc.tensor.matmul(
                                    prz_r, hw[:, 0:H], zh[:],
                                    start=False, stop=True))
                        else:
                            rz.append(lambda: nc.tensor.matmul(
                                prz_r, hw[:, 0:H], h_prev[:],
                                start=False, stop=True))
                    rz.append(lambda: nc.tensor.matmul(prz_z,
                                                       xw[:, H : 2 * H],
                                                       x_rhs, start=True,
                                                       stop=False if zbias_mm
                                                       else alone))
                    npx = []
                    if not alone:
                        rz.append(lambda: nc.tensor.matmul(
                            prz_z, hw[:, H : 2 * H], h_prev[:],
                            start=False, stop=not zbias_mm))
                        self.pn = pnxpool.tile([H, BL], F32, tag=f"pn{tag}")
                        pn = self.pn
                        npx.append(lambda: nc.tensor.matmul(
                            pn[:], hw[:, 2 * H :], h_prev[:],
                            start=True, stop=True))
                    if zbias_mm:
                        rz.append(lambda: nc.tensor.matmul(
                            prz_z, bz2row[:], onesr[:],
                            start=False, stop=True))
                    npx.append(lambda: nc.tensor.matmul(px[:],
                                                        xw[:, 2 * H :],
                                                        x_rhs, start=True,
                                                        stop=False))
                    self._rz, self._npx = rz, npx

                def emit_rz(self, n=None):
                    k = len(self._rz) if n is None else min(n, len(self._rz))
                    for t in self._rz[:k]:
                        t()
                    self._rz = self._rz[k:]

                def emit_npx(self):
                    for t in self._npx:
                        t()
                    self._npx = []

                def full_mms(self, x_rhs, h_prev):
                    self.stage_mms(x_rhs, h_prev)
                    self.emit_rz()
                    self.emit_npx()

                def sig(self, z_joint=False):
                    tag = self.tag
                    if self.rz_bias is None and CONFIG[f"rz{tag}_fused"]:
                        rz = work.tile([H, 2 * BL], DT, tag=f"rz{tag}")
                        nc.scalar.activation(rz[:], self.prz_rz, AF.Sigmoid)
                        self.r, self.z = rz[:, 0:BL], rz[:, BL:]
                        return
                    rb = dict(bias=self.rz_bias[0]) if self.rz_bias else {}
                    if CONFIG[f"sr{tag}_psum"]:
                        # sigmoid(r) written in-place over its own PSUM
                        # pre-activations: skips the SBUF write-access
                        # penalty (222 vs 172 cycles) on the critical path.
                        # t2 reads PSUM f32 either way (pn forces 1x mode).
                        nc.scalar.activation(self.prz_r, self.prz_r,
                                             AF.Sigmoid, **rb)
                        self.r = self.prz_r
                    else:
                        rt = work.tile([H, BL], DT, tag=f"r{tag}")
                        nc.scalar.activation(rt[:], self.prz_r,
                                             AF.Sigmoid, **rb)
                        self.r = rt[:]
                    if z_joint:
                        return  # z sigmoid emitted jointly via sig_zj()
                    # z bias: with szj it is already accumulated in PSUM by
                    # the rank-1 matmul, so no activation bias either way.
                    zb = ({} if (CONFIG["szj"] and tag == "2")
                          else (dict(bias=self.rz_bias[1]) if self.rz_bias
                                else {}))
                    zt = work.tile([H, BL], DT, tag=f"z{tag}")
                    nc.scalar.activation(zt[:], self.prz_z,
                                         AF.Sigmoid, **zb)
                    self.z = zt[:]

                def sig_zj(self, other):
                    # Joint sigmoid over [z1|z2] of the shared PSUM tile.
                    zj = work.tile([H, 2 * BL], DT, tag="zj")
                    nc.scalar.activation(zj[:], self.przJ[:, BL : 3 * BL],
                                         AF.Sigmoid)
                    self.z = zj[:, 0:BL]
                    other.z = zj[:, BL:]

                def prep(self, h_prev):
                    # Off-critical-path pieces of the h update:
                    # oz = 1 - z, zh = z * h_prev.
                    if not self.use_zh:
                        return
                    tag = self.tag
                    eng = ENG[CONFIG[f"prep{tag}_eng"]]
                    oz_eng = ENG[CONFIG[f"oz{tag}_eng"]]
                    self.oz = work.tile([H, BL], DT, tag=f"oz{tag}")
                    oz_eng.tensor_scalar(self.oz[:], self.z, -1.0, 1.0,
                                         op0=OP.mult, op1=OP.add)
                    if h_prev is not None:
                        self.zh = work.tile([H, BL], DT, tag=f"zh{tag}")
                        eng.tensor_mul(self.zh[:], self.z, h_prev[:])

                def t2(self, h_prev):
                    tag = self.tag
                    self.t2t = work.tile([H, BL], DT, tag=f"t2{tag}")
                    if h_prev is None:
                        ENG[CONFIG['t2_eng']].tensor_scalar_mul(
                            self.t2t[:], self.r, self.n_hh_bias)
                        return
                    # t2 = (hn + b_hh_n) * r
                    eng = ENG[CONFIG['t2_eng']]
                    if CONFIG["t2_split"]:
                        # Half-width wavefront: imm() starts on half 0 while
                        # half 1 is still on the DVE, shortening the
                        # t2 -> imm -> tanh chain segment.
                        hb = BL // 2
                        for s in (slice(0, hb), slice(hb, BL)):
                            eng.scalar_tensor_tensor(
                                self.t2t[:, s], self.pn[:, s], self.n_hh_bias,
                                self.r[:, s], op0=OP.add, op1=OP.mult)
                    else:
                        eng.scalar_tensor_tensor(
                            self.t2t[:], self.pn[:], self.n_hh_bias, self.r,
                            op0=OP.add, op1=OP.mult)

                def imm(self):
                    # px += I.T @ t2 on the PE, then tanh straight off PSUM
                    if CONFIG["t2_split"]:
                        hb = BL // 2
                        nc.tensor.matmul(self.px[:, 0:hb], ident[:],
                                         self.t2t[:, 0:hb], start=False,
                                         stop=True)
                        nc.tensor.matmul(self.px[:, hb:], ident[:],
                                         self.t2t[:, hb:], start=False,
                                         stop=True)
                    else:
                        nc.tensor.matmul(self.px[:], ident[:], self.t2t[:],
                                         start=False, stop=True)

                def tanh(self):
                    tag = self.tag
                    nb = dict(bias=self.n_ih_bias) if self.n_ih_bias is not None else {}
                    self.n = work.tile([H, BL], DT, tag=f"n{tag}")
                    nc.scalar.activation(self.n[:], self.px[:], AF.Tanh, **nb)

                def update(self, h_prev):
                    tag = self.tag
                    eng = ENG[CONFIG[f"upd{tag}_eng"]]
                    h_new = hpool.tile([H, BL], DT, tag=f"h{tag}")
                    if self.use_zh:
                        # h' = n*(1-z) + z*h  (2 hops after tanh)
                        if h_prev is None:
                            eng.tensor_mul(h_new[:], self.n[:], self.oz[:])
                            self.u_last, self.zh_last = h_new, None
                        else:
                            u = work.tile([H, BL], DT, tag=f"u{tag}")
                            eng.tensor_mul(u[:], self.n[:], self.oz[:])
                            eng.tensor_add(h_new[:], u[:], self.zh[:])
                            self.u_last, self.zh_last = u, self.zh
                    else:
                        # h' = n + z*(h - n)  (3 hops after tanh)
                        d = work.tile([H, BL], DT, tag=f"d{tag}")
                        if h_prev is not None:
                            nc.vector.tensor_sub(d[:], h_prev[:], self.n[:])
                        else:
                            nc.vector.tensor_scalar_mul(d[:], self.n[:], -1.0)
                        e = work.tile([H, BL], DT, tag=f"e{tag}")
                        nc.vector.tensor_mul(e[:], self.z, d[:])
                        nc.vector.tensor_add(h_new[:], self.n[:], e[:])
                    return h_new

            c1 = Cell("1", l1w, hh1w, None, sbias[:, 0:1], None,
                      CONFIG["zh1"])
            c2 = Cell("2", ih2w, hh2w, (sbias[:, 1:2], sbias[:, 2:3]),
                      sbias[:, 3:4], sbias[:, 4:5], CONFIG["zh2"])

            xc = None

            def xg(t):
                nonlocal xc
                if t % CHUNK == 0:
                    xc = xpool.tile([DA, CHUNK, BL], DT, tag="xc")
                    nc.sync.dma_start(
                        out=xc[:],
                        in_=xT_d[t : t + CHUNK].rearrange("t d b -> d t b"))
                return xc[:, t % CHUNK, :]

            lag = CONFIG["l2_lag"]
            for _rep in range(repeats):
              # Prologue: layer-1 step 0 alone.
              c1.in_mms(xg(0), alone=True)
              c1.sig()
              c1.prep(None)
              c1.t2(None)
              c1.imm()
              c1.tanh()
              h1_hist = {0: c1.update(None)}
              if n_steps > 1:
                c1.in_mms(xg(1))

              h2_prev = None
              pending2 = None       # deferred L2 tail: h2_in of that step
              have_pending = False
              for i in range(1, n_steps):
                  # Layer 1 works on step i; layer 2 on step j = i - lag.
                  j = i - lag
                  c2_on = j >= 0
                  h1_prev = h1_hist[i - 1]
                  zj_on = CONFIG["szj"] and c2_on
                  c1.rec_mms(h1_prev)
                  c1.sig(z_joint=zj_on)
                  c1.t2(h1_prev)        # DVE-first: t21 gates the chain
                  if have_pending:
                      # L2 tail of the previous iteration's step: tanh2 here
                      # fills the Act window instead of queue-blocking the
                      # next sr1 at the previous iteration's end.
                      c2.tanh()
                      h2_prev = c2.update(pending2)
                      have_pending = False
                  h2_in = h2_prev if (c2_on and j > 0) else None
                  if not zj_on:
                      c1.prep(h1_prev)  # oz1 (+ zh1, possibly on Pool)
                  if c2_on:
                      c2.stage_mms(h1_hist[j], h2_in)
                      c2.emit_rz(CONFIG["imm1_pos"])
                  c1.imm()              # PE: placed imm1_pos deep into L2's
                  if c2_on:             # rz matmuls so it runs when t21 lands
                      c2.emit_rz()
                      if CONFIG["tanh1_before_sig2"]:
                          c1.tanh()
                      if zj_on and CONFIG["szj_first"]:
                          c1.sig_zj(c2)
                          c1.prep(h1_prev)
                      c2.sig(z_joint=zj_on)
                      if zj_on and not CONFIG["szj_first"]:
                          c1.sig_zj(c2)
                          c1.prep(h1_prev)
                      if not CONFIG["tanh1_before_sig2"]:
                          c1.tanh()
                      c2.emit_npx()
                      if not CONFIG["t22_late"]:
                          c2.t2(h2_in)
                          c2.imm()
                      c2.prep(h2_in)
                  else:
                      c1.tanh()
                  h1_new = c1.update(h1_prev)
                  if c2_on and CONFIG["t22_late"]:
                      c2.t2(h2_in)
                      c2.imm()
                  if i + 1 < n_steps:
                      c1.in_mms(xg(i + 1))
                  if c2_on:
                      if CONFIG["tail2_defer"]:
                          pending2 = h2_in
                          have_pending = True
                      else:
                          c2.tanh()
                          h2_prev = c2.update(h2_in)
                  h1_hist[i] = h1_new

              if have_pending:
                  c2.tanh()
                  h2_prev = c2.update(pending2)
                  have_pending = False

              # Epilogue: remaining layer-2 steps.
              for j in range(max(n_steps - lag, 0), n_steps):
                  h2_in = h2_prev if j > 0 else None
                  c2.full_mms(h1_hist[j], h2_in)
                  c2.sig()
                  c2.prep(h2_in)
                  c2.t2(h2_in)
                  c2.imm()
                  c2.tanh()
                  h2_prev = c2.update(h2_in)

              # ---------------- FC head ------------------------------------
              pfc = pnxpool.tile([NCLS, BL], F32, tag="pn1")
              nc.tensor.matmul(pfc[:], fcw[:], h2_prev[:], start=True, stop=True)
              outs = work.tile([NCLS, BL], F32, tag="outs")
              nc.scalar.activation(outs[:], pfc[:], AF.Identity, bias=fcb[:])
              nc.sync.dma_start(out=out_d[:], in_=outs[:])

    if not for_sim:
        _split_excess_waits(nc)
    return nc


def prep_weights(inputs: dict) -> dict:
    """Pack the small GRU/FC weights into the kernel's layouts (host numpy)."""
    w_ih1 = np.asarray(inputs["w_ih1"], np.float32)
    w_hh1 = np.asarray(inputs["w_hh1"], np.float32)
    b_ih1 = np.asarray(inputs["b_ih1"], np.float32)
    b_hh1 = np.asarray(inputs["b_hh1"], np.float32)
    w_ih2 = np.asarray(inputs["w_ih2"], np.float32)
    w_hh2 = np.asarray(inputs["w_hh2"], np.float32)
    b_ih2 = np.asarray(inputs["b_ih2"], np.float32)
    b_hh2 = np.asarray(inputs["b_hh2"], np.float32)
    fc_w = np.asarray(inputs["fc_w"], np.float32)
    fc_b = np.asarray(inputs["fc_b"], np.float32)

    # layer-1 combined input-proj weights + bias row.
    # r/z columns carry b_ih1+b_hh1; n columns carry b_ih1 only (b_hh1_n must
    # be applied inside r*(hn+b_hh1_n)).
    l1w = np.empty((DA, G3), np.float32)
    l1w[0:D, :] = w_ih1.T
    bias_row = b_ih1.copy()
    bias_row[0 : 2 * H] += b_hh1[0 : 2 * H]
    l1w[D, :] = bias_row

    bvec = np.stack(
        [
            b_hh1[2 * H : 3 * H],                     # col 0: L1 n-gate hh bias
            (b_ih2 + b_hh2)[0:H],                     # col 1: L2 r bias
            (b_ih2 + b_hh2)[H : 2 * H],               # col 2: L2 z bias
            b_hh2[2 * H : 3 * H],                     # col 3: L2 n-gate hh bias
            b_ih2[2 * H : 3 * H],                     # col 4: L2 n-gate ih bias
        ],
        axis=1,
    ).astype(np.float32)

    if CONFIG["dtype"] == "bf16":
        import ml_dtypes
        ndt = np.dtype(ml_dtypes.bfloat16)
    else:
        ndt = _NP_DT[CONFIG["dtype"]]
    return {
        "l1w": np.ascontiguousarray(l1w).astype(ndt),
        "hh1w": np.ascontiguousarray(w_hh1.T).astype(ndt),
        "ih2w": np.ascontiguousarray(w_ih2.T).astype(ndt),
        "hh2w": np.ascontiguousarray(w_hh2.T).astype(ndt),
        "bvec": bvec,
        "fcw": np.ascontiguousarray(fc_w.T).astype(ndt),
        "fcb": np.ascontiguousarray(fc_b[:, None]),
        "ident": np.eye(H, dtype=np.float32).astype(ndt),
        # szj: L2 z-gate bias as a rank-1 matmul (bz2row.T @ onesr)
        "bz2row": np.ascontiguousarray(
            (b_ih2 + b_hh2)[H : 2 * H][None, :]).astype(ndt),
        "onesr": np.ones((1, BL), np.float32).astype(ndt),
    }


_EXEC = {}


def get_executor(repeats: int = 1):
    """Build (once per `repeats`) the jitted shard_map callable around the
    bass program.

    The jit body contains ONLY the bass_exec custom call (the neuronx-cc
    hook rejects modules with extra computations), so all input repacking
    happens host-side and the argument arrays are staged on device by
    device_inputs(). repeats > 1 builds a timing variant that runs the
    whole forward pass that many times back-to-back on-device.
    """
    if repeats in _EXEC:
        return _EXEC[repeats]

    import jax
    from jax.experimental.shard_map import shard_map
    from jax.sharding import Mesh, PartitionSpec, NamedSharding
    from concourse import bass2jax

    bass2jax.install_neuronx_cc_hook()

    nc = build_program(repeats=repeats)
    partition_name = nc.partition_id_tensor.name if nc.partition_id_tensor else None
    in_names, out_names, out_avals = [], [], []
    for alloc in nc.m.functions[0].allocations:
        if not isinstance(alloc, mybir.MemoryLocationSet):
            continue
        name = alloc.memorylocations[0].name
        if alloc.kind == "ExternalInput":
            if name != partition_name:
                in_names.append(name)
        elif alloc.kind == "ExternalOutput":
            shape = tuple(alloc.tensor_shape)
            dtype = mybir.dt.np(alloc.dtype)
            out_names.append(name)
            out_avals.append(jax.core.ShapedArray(shape, dtype))
    all_in_names = list(in_names) + list(out_names)
    if partition_name is not None:
        all_in_names.append(partition_name)

    def _body(*args):
        operands = list(args)
        if partition_name is not None:
            operands.append(bass2jax.partition_id_tensor())
        outs = bass2jax._bass_exec_p.bind(
            *operands,
            out_avals=tuple(out_avals),
            in_names=tuple(all_in_names),
            out_names=tuple(out_names),
            lowering_input_output_aliases=(),
            sim_require_finite=True,
            sim_require_nnan=True,
            nc=nc,
        )
        return tuple(outs)

    devices = jax.devices()[:N_CORES]
    mesh = Mesh(np.asarray(devices), ("core",))
    spec = PartitionSpec("core")
    n_args = len(in_names) + len(out_avals)
    sharded = jax.jit(
        shard_map(
            _body,
            mesh=mesh,
            in_specs=(spec,) * n_args,
            out_specs=(spec,) * len(out_avals),
            check_rep=False,
        )
    )
    _EXEC[repeats] = {
        "fn": sharded,
        "mesh": mesh,
        "sharding": NamedSharding(mesh, spec),
        "in_names": in_names,
        "out_names": out_names,
        "out_avals": out_avals,
        "nc": nc,
    }
    return _EXEC[repeats]


def prep_host(inputs: dict) -> dict:
    """Host-side repack of the full inputs into per-core concatenated
    arrays keyed by DRAM tensor name (axis 0 = core for shard_map)."""
    x = np.asarray(inputs["x"])
    xr = x.astype(np.float16).reshape(N_CORES, BL, 2, L, D // 2)
    xt = xr.transpose(0, 3, 2, 4, 1).reshape(N_CORES, L, D, BL)
    xT = np.concatenate(
        [xt, np.ones((N_CORES, L, 1, BL), np.float16)], axis=2)

    w = prep_weights(inputs)
    arrs = {"xT": xT.reshape(N_CORES * L, DA, BL)}
    for name, val in w.items():
        arrs[name] = np.broadcast_to(
            val, (N_CORES,) + val.shape).reshape((N_CORES * val.shape[0],)
                                                + val.shape[1:])
    return arrs


def device_inputs(inputs: dict):
    """Host prep + H2D: the jit's argument list, already sharded on the
    mesh. Output operands are staged zero buffers (the kernel writes every
    element of outT, so they can be reused across calls; not donated)."""
    import jax

    ex = get_executor(1)
    arrs = prep_host(inputs)
    args = [arrs[n] for n in ex["in_names"]]
    for a in ex["out_avals"]:
        args.append(np.zeros((N_CORES * a.shape[0],) + tuple(a.shape[1:]),
                             a.dtype))
    return [jax.device_put(a, ex["sharding"]) for a in args]


def assemble_output(outs) -> np.ndarray:
    # outT concat over cores: (8*24, 512) -> (4096, 24)
    outT = np.asarray(outs[0])
    return np.ascontiguousarray(
        outT.reshape(N_CORES, NCLS, BL).transpose(0, 2, 1).reshape(B_TOT, NCLS)
    ).astype(np.float32)


def kernel(**inputs) -> np.ndarray:
    import jax

    ex = get_executor(1)
    args = device_inputs(inputs)
    outs = ex["fn"](*args)
    jax.block_until_ready(outs)
    return assemble_output(outs)

